# revision 1
# baseline (speedup 1.0000x reference)
"""GQA attention (RoPE + ALiBi + causal) on 8 trn2 NeuronCores.

Sharding: core c -> batch b = c//4, kv-group g = c%4 (4 q-heads + 1 kv-head
per core, column-sharded Wq/Wk/Wv, row-sharded Wo; host sums the 4 partial
Wo outputs per batch).

All device matmuls run in float32r (full-rate PE) with N=512 moving dims.
Everything is kept transposed ([feature, token]) so softmax reductions over
keys become partition-dim reductions done with ones-vector matmuls, and the
per-key ALiBi column bias rides the exp() activation's per-partition bias.
The per-query ALiBi term is added with a K=1 ones matmul into the same PSUM
accumulation. Causal structure: only lower-triangle key tiles are computed;
diagonal tiles get the (transposed) mask block added before exp.
"""
import sys

if '/opt/trn_rl_repo' not in sys.path:
    sys.path.insert(0, '/opt/trn_rl_repo')

import numpy as np

B, T, D = 2, 2048, 2048
H, KV = 16, 4
HD = D // H          # 128
NREP = H // KV       # 4
KVD = 512            # per-core q width (4 heads x 128)
P = 128
TB = 512             # t-block
NBLK = T // TB       # 4
NC = D // P          # 16 contraction tiles
NJ = T // P          # 16 key tiles
ALIBI_W = 0.1
SCALE = (1.0 - ALIBI_W) / np.sqrt(np.float32(HD))

_cache = {}


def _build():
    from concourse import bacc, mybir
    from concourse.tile import TileContext

    F32 = mybir.dt.float32
    FR = mybir.dt.float32r
    EXP = mybir.ActivationFunctionType.Exp

    nc = bacc.Bacc()
    xT = nc.declare_dram_parameter("xT", [D, T], F32, isOutput=False)
    wq = nc.declare_dram_parameter("wq", [D, KVD], F32, isOutput=False)
    wk = nc.declare_dram_parameter("wk", [D, P], F32, isOutput=False)
    wv = nc.declare_dram_parameter("wv", [D, P], F32, isOutput=False)
    wo = nc.declare_dram_parameter("wo", [KVD, D], F32, isOutput=False)
    cosq = nc.declare_dram_parameter("cosq", [P, T], F32, isOutput=False)
    sinq = nc.declare_dram_parameter("sinq", [P, T], F32, isOutput=False)
    cosk = nc.declare_dram_parameter("cosk", [P, T], F32, isOutput=False)
    sink = nc.declare_dram_parameter("sink", [P, T], F32, isOutput=False)
    cb = nc.declare_dram_parameter("cb", [P, NREP * NBLK * NJ], F32, isOutput=False)
    maskT = nc.declare_dram_parameter("maskT", [P, 4 * TB], F32, isOutput=False)
    onesc = nc.declare_dram_parameter("onesc", [P, 1], F32, isOutput=False)
    idin = nc.declare_dram_parameter("idin", [P, P], F32, isOutput=False)
    out = nc.declare_dram_parameter("out", [T, D], F32, isOutput=True)

    with TileContext(nc) as tc:
        with (
            tc.tile_pool(name="const", bufs=1) as cpool,
            tc.tile_pool(name="kv", bufs=1) as kvpool,
            tc.tile_pool(name="tabs", bufs=1) as tpool,
            tc.tile_pool(name="xin", bufs=3) as xpool,
            tc.tile_pool(name="work", bufs=2) as wpool,
            tc.tile_pool(name="qt", bufs=4) as qpool,
            tc.tile_pool(name="pt", bufs=3) as ptpool,
            tc.tile_pool(name="ot", bufs=4) as opool,
            tc.tile_pool(name="ysb", bufs=2) as ypool,
            tc.tile_pool(name="small", bufs=2) as spool,
            tc.tile_pool(name="ps", bufs=1, space="PSUM") as pss,
        ):
            # ---- resident constants ----
            wq_sb = cpool.tile([P, NC, KVD], FR)
            wq_r = wq.rearrange("(c p) n -> p c n", p=P).bitcast(FR)
            for c in range(NC):
                nc.sync.dma_start(out=wq_sb[:, c], in_=wq_r[:, c])
            wk_sb = cpool.tile([P, NC, P], FR)
            wk_r = wk.rearrange("(c p) n -> p c n", p=P).bitcast(FR)
            wv_sb = cpool.tile([P, NC, P], FR)
            wv_r = wv.rearrange("(c p) n -> p c n", p=P).bitcast(FR)
            for c4 in range(4):
                nc.sync.dma_start(out=wk_sb[:, c4 * 4:(c4 + 1) * 4], in_=wk_r[:, c4 * 4:(c4 + 1) * 4])
                nc.sync.dma_start(out=wv_sb[:, c4 * 4:(c4 + 1) * 4], in_=wv_r[:, c4 * 4:(c4 + 1) * 4])
            wo_sb = cpool.tile([P, NREP, D], FR)
            wo_r = wo.rearrange("(h p) e -> p h e", p=P).bitcast(FR)
            for h in range(NREP):
                nc.sync.dma_start(out=wo_sb[:, h], in_=wo_r[:, h])
            cb_sb = cpool.tile([P, NREP * NBLK * NJ], F32)
            nc.sync.dma_start(out=cb_sb, in_=cb[:, :])
            maskT_sb = cpool.tile([P, 4 * TB], F32)
            nc.sync.dma_start(out=maskT_sb, in_=maskT[:, :])
            onesc_sb = cpool.tile([P, 1], FR)
            nc.sync.dma_start(out=onesc_sb, in_=onesc[:, :].bitcast(FR))
            id_sb = cpool.tile([P, P], FR)
            nc.sync.dma_start(out=id_sb, in_=idin[:, :].bitcast(FR))

            kT_sb = kvpool.tile([P, T], FR)          # roped K, [d, s]
            v_sb = kvpool.tile([P, NJ, P], FR)       # V tiles, [s, j, d']

            for bk in range(NBLK):
                t0 = bk * TB
                # ---- tables for this block ----
                cq_t = tpool.tile([P, TB], F32, tag="cq")
                nc.sync.dma_start(out=cq_t, in_=cosq[:, t0:t0 + TB])
                sq_t = tpool.tile([P, TB], F32, tag="sq")
                nc.sync.dma_start(out=sq_t, in_=sinq[:, t0:t0 + TB])
                ck_t = tpool.tile([P, TB], F32, tag="ck")
                nc.sync.dma_start(out=ck_t, in_=cosk[:, t0:t0 + TB])
                sk_t = tpool.tile([P, TB], F32, tag="sk")
                nc.sync.dma_start(out=sk_t, in_=sink[:, t0:t0 + TB])

                # ---- projections ----
                q_ps = [None] * NREP
                q_ps[0] = pss.tile([P, TB], F32, tag="big", bufs=7, name=f"qps{bk}_0")
                q_ps[1] = pss.tile([P, TB], F32, tag="big", bufs=7, name=f"qps{bk}_1")
                k_ps = pss.tile([P, TB], F32, tag="big", bufs=7, name=f"kps{bk}")
                v_ps = pss.tile([P, TB], F32, tag="big", bufs=7, name=f"vps{bk}")
                for c in range(NC):
                    xt = xpool.tile([P, TB], FR, tag="xt", name=f"xtA{bk}_{c}")
                    nc.sync.dma_start(out=xt, in_=xT[c * P:(c + 1) * P, t0:t0 + TB].bitcast(FR))
                    for h in (0, 1):
                        nc.tensor.matmul(q_ps[h], wq_sb[:, c, h * P:(h + 1) * P], xt,
                                         start=(c == 0), stop=(c == NC - 1))
                    nc.tensor.matmul(k_ps, wk_sb[:, c, :], xt, start=(c == 0), stop=(c == NC - 1))
                    nc.tensor.matmul(v_ps, wv_sb[:, c, :], xt, start=(c == 0), stop=(c == NC - 1))
                q_ps[2] = pss.tile([P, TB], F32, tag="big", bufs=7, name=f"qps{bk}_2")
                q_ps[3] = pss.tile([P, TB], F32, tag="big", bufs=7, name=f"qps{bk}_3")
                for c in range(NC):
                    xt = xpool.tile([P, TB], FR, tag="xt", name=f"xtB{bk}_{c}")
                    nc.sync.dma_start(out=xt, in_=xT[c * P:(c + 1) * P, t0:t0 + TB].bitcast(FR))
                    for h in (2, 3):
                        nc.tensor.matmul(q_ps[h], wq_sb[:, c, h * P:(h + 1) * P], xt,
                                         start=(c == 0), stop=(c == NC - 1))

                # ---- RoPE ----
                def rope(dst, src_ps, cos_t, sin_t, nm):
                    raw = wpool.tile([P, TB], F32, tag="raw", name=f"raw{nm}")
                    nc.scalar.copy(raw, src_ps)
                    swp = wpool.tile([P, TB], F32, tag="swp", name=f"swp{nm}")
                    nc.sync.dma_start(out=swp[0:64, :], in_=raw[64:128, :])
                    nc.sync.dma_start(out=swp[64:128, :], in_=raw[0:64, :])
                    m1 = wpool.tile([P, TB], F32, tag="m1", name=f"m1{nm}")
                    nc.vector.tensor_mul(m1, src_ps, cos_t)
                    m2 = wpool.tile([P, TB], F32, tag="m2", name=f"m2{nm}")
                    nc.vector.tensor_mul(m2, swp, sin_t)
                    nc.vector.tensor_add(dst, m1, m2)

                q_sb = []
                for h in range(NREP):
                    qh = qpool.tile([P, TB], FR, tag="qT", name=f"qT{bk}_{h}")
                    rope(qh, q_ps[h], cq_t, sq_t, f"q{bk}_{h}")
                    q_sb.append(qh)
                rope(kT_sb[:, t0:t0 + TB], k_ps, ck_t, sk_t, f"k{bk}")

                # ---- V: copy + transpose to [s, d'] ----
                vtmp = wpool.tile([P, TB], FR, tag="vtmp", name=f"vtmp{bk}")
                nc.scalar.copy(vtmp, v_ps)
                for sj in range(4):
                    vt_ps = pss.tile([P, P], FR, tag="big", bufs=7, name=f"vtps{bk}_{sj}")
                    nc.tensor.transpose(vt_ps, vtmp[:, sj * P:(sj + 1) * P], id_sb)
                    nc.vector.tensor_copy(v_sb[:, 4 * bk + sj, :], vt_ps)

                # ---- attention ----
                nj = 4 * bk + 4
                for h in range(NREP):
                    ot_ps = pss.tile([P, TB], F32, tag="big", bufs=7, name=f"otps{bk}_{h}")
                    cs_ps = pss.tile([1, TB], F32, tag="cs", bufs=1, name=f"csps{bk}_{h}")
                    for j in range(nj):
                        s_ps = pss.tile([P, TB], F32, tag="big", bufs=7, name=f"sps{bk}_{h}_{j}")
                        nc.tensor.matmul(s_ps, kT_sb[:, j * P:(j + 1) * P], q_sb[h],
                                         start=True, stop=True)
                        delta = j - 4 * bk
                        if delta >= 0:
                            nc.vector.tensor_add(s_ps, s_ps,
                                                 maskT_sb[:, delta * TB:(delta + 1) * TB])
                        pt = ptpool.tile([P, TB], FR, tag="pt", name=f"pt{bk}_{h}_{j}")
                        nc.scalar.activation(pt, s_ps, EXP,
                                             bias=cb_sb[:, (h * NBLK + bk) * NJ + j:(h * NBLK + bk) * NJ + j + 1])
                        nc.tensor.matmul(cs_ps, onesc_sb, pt,
                                         start=(j == 0), stop=(j == nj - 1))
                        nc.tensor.matmul(ot_ps, v_sb[:, j, :], pt,
                                         start=(j == 0), stop=(j == nj - 1))
                    rec = spool.tile([1, TB], F32, tag="rec", name=f"rec{bk}_{h}")
                    nc.vector.reciprocal(rec, cs_ps)
                    rbc = spool.tile([P, TB], F32, tag="rbc", name=f"rbc{bk}_{h}")
                    nc.gpsimd.partition_broadcast(rbc, rec)
                    oh = opool.tile([P, TB], FR, tag="ot", name=f"ot{bk}_{h}")
                    nc.vector.tensor_mul(oh, ot_ps, rbc)
                    q_sb[h] = oh  # reuse list slot to keep handles

                ot_sb = q_sb  # [h] -> [d', t] normalized attention out

                # ---- Wo partial ----
                for ts_ in range(4):
                    for e in range(4):
                        y_ps = pss.tile([P, TB], F32, tag="big", bufs=7, name=f"yps{bk}_{ts_}_{e}")
                        for h in range(NREP):
                            nc.tensor.matmul(y_ps, ot_sb[h][:, ts_ * P:(ts_ + 1) * P],
                                             wo_sb[:, h, e * TB:(e + 1) * TB],
                                             start=(h == 0), stop=(h == NREP - 1))
                        y_sb = ypool.tile([P, TB], F32, tag="ysb", name=f"y{bk}_{ts_}_{e}")
                        nc.vector.tensor_copy(y_sb, y_ps)
                        nc.sync.dma_start(
                            out=out[t0 + ts_ * P:t0 + (ts_ + 1) * P, e * TB:(e + 1) * TB],
                            in_=y_sb)

    nc.compile()
    return nc


def _prep_inputs(x, mask, freqs_cis, alibi_bias, Wq, Wk, Wv, Wo):
    """Host-side prep: transposes, RoPE tables, ALiBi bias decomposition."""
    f64 = np.float64
    idx = np.arange(HD)
    cos_full = freqs_cis[:, idx // 2]                     # [T, 128]
    sin_full = freqs_cis[:, (HD // 2) + idx // 2]         # [T, 128]
    sign = np.where(idx < HD // 2, -1.0, 1.0).astype(np.float32)
    cosT = np.ascontiguousarray(cos_full.T)               # [128, T]
    sinT_signed = np.ascontiguousarray((sin_full * sign[None, :]).T)

    cosq = (cosT * np.float32(SCALE)).astype(np.float32)
    sinq = (sinT_signed * np.float32(SCALE)).astype(np.float32)
    cosk = cosT.astype(np.float32)
    sink = sinT_signed.astype(np.float32)

    m = mask[0, 0]
    maskT = np.empty((P, 4 * TB), np.float32)
    for d in range(4):
        maskT[:, d * TB:(d + 1) * TB] = m[:TB, d * P:(d + 1) * P].T

    onesc = np.ones((P, 1), np.float32)
    idin = np.eye(P, dtype=np.float32)

    in_maps = []
    for c in range(8):
        b, g = c // 4, c % 4
        slopes = np.array([-f64(alibi_bias[0, g * NREP + hl, 1, 0]) for hl in range(NREP)])
        pvec = np.arange(P, dtype=f64)
        jvec = np.arange(NJ, dtype=f64)
        # cb[p, h, bk, j] = ALIBI_W*slope*(j*128 + p) - ALIBI_W*slope*(bk*512 + 511)
        bkvec = np.arange(NBLK, dtype=f64)
        cbv = (ALIBI_W * slopes[:, None, None, None]
               * (jvec[None, None, :, None] * P + pvec[None, None, None, :]
                  - (bkvec[None, :, None, None] * TB + (TB - 1))))
        cbm = np.ascontiguousarray(cbv.transpose(3, 0, 1, 2).reshape(P, NREP * NBLK * NJ)).astype(np.float32)
        in_maps.append({
            "xT": np.ascontiguousarray(x[b].T),
            "wq": np.ascontiguousarray(Wq[:, g * KVD:(g + 1) * KVD]),
            "wk": np.ascontiguousarray(Wk[:, g * P:(g + 1) * P]),
            "wv": np.ascontiguousarray(Wv[:, g * P:(g + 1) * P]),
            "wo": np.ascontiguousarray(Wo[g * KVD:(g + 1) * KVD, :]),
            "cosq": cosq, "sinq": sinq, "cosk": cosk, "sink": sink,
            "cb": cbm, "maskT": maskT,
            "onesc": onesc, "idin": idin,
        })
    return in_maps


def kernel(x, mask, freqs_cis, alibi_bias, Wq, Wk, Wv, Wo, _trace=False, _trace_kwargs=None):
    from concourse.bass_utils import run_bass_kernel_spmd

    if "nc" not in _cache:
        _cache["nc"] = _build()
    nc = _cache["nc"]

    in_maps = _prep_inputs(np.asarray(x, np.float32), np.asarray(mask, np.float32),
                           np.asarray(freqs_cis, np.float32), np.asarray(alibi_bias, np.float32),
                           np.asarray(Wq, np.float32), np.asarray(Wk, np.float32),
                           np.asarray(Wv, np.float32), np.asarray(Wo, np.float32))
    kw = {}
    if _trace:
        kw = dict(trace=True, **(_trace_kwargs or {}))
    res = run_bass_kernel_spmd(nc, in_maps, list(range(8)), **kw)

    full = np.zeros((B, T, D), np.float32)
    for c in range(8):
        full[c // 4] += res.results[c]["out"]
    if _trace:
        _cache["last_trace"] = res
    return full



# revision 5
# speedup vs baseline: 1.8524x; 1.8524x over previous
"""GQA attention (RoPE + ALiBi + causal) on 8 trn2 NeuronCores.

Sharding: core c -> batch b = c//4, kv-group g = c%4 (4 q-heads + 1 kv-head
per core, column-sharded Wq/Wk/Wv, row-sharded Wo; host sums the 4 partial
Wo outputs per batch).

All matmuls run in bf16 (f32 PSUM accumulation). Everything is kept
transposed ([feature, token]) so softmax reductions over keys become
partition-dim reductions done with ones-vector matmuls, and the per-key
ALiBi column bias rides the exp() activation's per-partition bias (the
per-query ALiBi part is constant per token and cancels in softmax up to
the per-block offset baked into the bias). Causal structure: only
lower-triangle key tiles are computed; diagonal tiles are masked with a
multiplicative {0,1} mask after exp. V is projected directly transposed
(x-tile stationary). The emission order interleaves next-block
projections between attention and Wo so the PE never idles long enough
for HAM to re-throttle.
"""
import sys

if '/opt/trn_rl_repo' not in sys.path:
    sys.path.insert(0, '/opt/trn_rl_repo')

import numpy as np
import ml_dtypes

BFNP = ml_dtypes.bfloat16

B, T, D = 2, 2048, 2048
H, KV = 16, 4
HD = D // H          # 128
NREP = H // KV       # 4
KVD = 512            # per-core q width (4 heads x 128)
P = 128
TB = 512             # t-block
NBLK = T // TB       # 4
NC = D // P          # 16 contraction tiles
NJ = T // P          # 16 key tiles
ALIBI_W = 0.1
SCALE = (1.0 - ALIBI_W) / np.sqrt(np.float32(HD))

# stream_shuffle: rotate partitions by 64 (16 groups of 4) - self-inverse
SHUF_MASK = [(g + 16) % 32 for g in range(32)]

_cache = {}


def _build():
    from concourse import bacc, mybir
    from concourse.tile import TileContext

    F32 = mybir.dt.float32
    BF16 = mybir.dt.bfloat16
    EXP = mybir.ActivationFunctionType.Exp

    nc = bacc.Bacc()
    xT = nc.declare_dram_parameter("xT", [D, T], BF16, isOutput=False)
    wq = nc.declare_dram_parameter("wq", [D, KVD], BF16, isOutput=False)
    wk = nc.declare_dram_parameter("wk", [D, P], BF16, isOutput=False)
    wv = nc.declare_dram_parameter("wv", [D, P], BF16, isOutput=False)
    wo = nc.declare_dram_parameter("wo", [KVD, D], BF16, isOutput=False)
    cosq = nc.declare_dram_parameter("cosq", [P, T], BF16, isOutput=False)
    sinq = nc.declare_dram_parameter("sinq", [P, T], BF16, isOutput=False)
    cosk = nc.declare_dram_parameter("cosk", [P, T], BF16, isOutput=False)
    sink = nc.declare_dram_parameter("sink", [P, T], BF16, isOutput=False)
    cb = nc.declare_dram_parameter("cb", [P, NREP * NBLK * NJ], F32, isOutput=False)
    m01 = nc.declare_dram_parameter("m01", [P, 4 * TB], BF16, isOutput=False)
    onesc = nc.declare_dram_parameter("onesc", [P, 1], BF16, isOutput=False)
    out = nc.declare_dram_parameter("out", [T, D], BF16, isOutput=True)

    with TileContext(nc) as tc:
        with (
            tc.tile_pool(name="const", bufs=1) as cpool,
            tc.tile_pool(name="kv", bufs=1) as kvpool,
            tc.tile_pool(name="xin", bufs=2) as xpool,
            tc.tile_pool(name="work", bufs=2) as wpool,
            tc.tile_pool(name="qt", bufs=8) as qpool,
            tc.tile_pool(name="pt", bufs=4) as ptpool,
            tc.tile_pool(name="ot", bufs=8) as opool,
            tc.tile_pool(name="ysb", bufs=2) as ypool,
            tc.tile_pool(name="small", bufs=2) as spool,
            tc.tile_pool(name="ps", bufs=1, space="PSUM") as pss,
        ):
            # ---- resident constants ----
            wq_sb = cpool.tile([P, NC, KVD], BF16)
            wq_r = wq.rearrange("(c p) n -> p c n", p=P)
            for c4 in range(4):
                nc.sync.dma_start(out=wq_sb[:, c4 * 4:(c4 + 1) * 4], in_=wq_r[:, c4 * 4:(c4 + 1) * 4])
            wk_sb = cpool.tile([P, NC, P], BF16)
            wk_r = wk.rearrange("(c p) n -> p c n", p=P)
            nc.sync.dma_start(out=wk_sb, in_=wk_r)
            wv_sb = cpool.tile([P, NC, P], BF16)
            wv_r = wv.rearrange("(c p) n -> p c n", p=P)
            nc.sync.dma_start(out=wv_sb, in_=wv_r)
            wo_sb = cpool.tile([P, NREP, D], BF16)
            wo_r = wo.rearrange("(h p) e -> p h e", p=P)
            for h in range(NREP):
                nc.sync.dma_start(out=wo_sb[:, h], in_=wo_r[:, h])
            cq_sb = cpool.tile([P, T], BF16)
            nc.sync.dma_start(out=cq_sb, in_=cosq[:, :])
            sq_sb = cpool.tile([P, T], BF16)
            nc.sync.dma_start(out=sq_sb, in_=sinq[:, :])
            ck_sb = cpool.tile([P, T], BF16)
            nc.sync.dma_start(out=ck_sb, in_=cosk[:, :])
            sk_sb = cpool.tile([P, T], BF16)
            nc.sync.dma_start(out=sk_sb, in_=sink[:, :])
            cb_sb = cpool.tile([P, NREP * NBLK * NJ], F32)
            nc.sync.dma_start(out=cb_sb, in_=cb[:, :])
            m01_sb = cpool.tile([P, 4 * TB], BF16)
            nc.sync.dma_start(out=m01_sb, in_=m01[:, :])
            ones_sb = cpool.tile([P, 1], BF16)
            nc.sync.dma_start(out=ones_sb, in_=onesc[:, :])

            kT_sb = kvpool.tile([P, T], BF16)        # roped K, [d, s]
            v_sb = kvpool.tile([P, NJ * P], BF16)    # V tiles, [s, j*d']

            xTr = xT.rearrange("(c p) t -> p c t", p=P)

            qh_l = [[None] * NREP for _ in range(NBLK)]
            oh_l = [[None] * NREP for _ in range(NBLK)]

            def rope(dst, src_ps, cos_ap, sin_ap, nm):
                raw = wpool.tile([P, TB], BF16, tag="raw", name=f"raw{nm}")
                nc.scalar.copy(raw, src_ps)
                swp = wpool.tile([P, TB], BF16, tag="swp", name=f"swp{nm}")
                nc.vector.stream_shuffle(swp, raw, SHUF_MASK)
                m1 = wpool.tile([P, TB], BF16, tag="m1", name=f"m1{nm}")
                nc.vector.tensor_mul(m1, raw, cos_ap)
                m2 = wpool.tile([P, TB], BF16, tag="m2", name=f"m2{nm}")
                nc.vector.tensor_mul(m2, swp, sin_ap)
                nc.vector.tensor_add(dst, m1, m2)

            def proj(bk):
                t0 = bk * TB
                xt = xpool.tile([P, NC, TB], BF16, tag="xt", name=f"xt{bk}")
                nc.sync.dma_start(out=xt[:, 0:8], in_=xTr[:, 0:8, t0:t0 + TB])
                nc.sync.dma_start(out=xt[:, 8:16], in_=xTr[:, 8:16, t0:t0 + TB])
                # q waves (one PSUM tile each)
                for h in range(NREP):
                    qp = pss.tile([P, TB], F32, tag="work", bufs=2, name=f"qp{bk}_{h}")
                    for c in range(NC):
                        nc.tensor.matmul(qp, wq_sb[:, c, h * P:(h + 1) * P], xt[:, c],
                                         start=(c == 0), stop=(c == NC - 1))
                    qh = qpool.tile([P, TB], BF16, tag="qh", name=f"qh{bk}_{h}")
                    rope(qh, qp, cq_sb[:, t0:t0 + TB], sq_sb[:, t0:t0 + TB], f"q{bk}{h}")
                    qh_l[bk][h] = qh
                # k wave
                kp = pss.tile([P, TB], F32, tag="work", bufs=2, name=f"kp{bk}")
                for c in range(NC):
                    nc.tensor.matmul(kp, wk_sb[:, c], xt[:, c], start=(c == 0), stop=(c == NC - 1))
                rope(kT_sb[:, t0:t0 + TB], kp, ck_sb[:, t0:t0 + TB], sk_sb[:, t0:t0 + TB], f"k{bk}")
                # v wave: directly transposed ([tok, d']) via x-tile-stationary
                # matmuls; one PSUM tile per token slice (a start=True matmul
                # clears has_written for its whole bank, so accumulation groups
                # must not share a bank)
                for ts_ in range(4):
                    vtp = pss.tile([P, P], F32, tag="work", bufs=2, name=f"vtp{bk}_{ts_}")
                    for c in range(NC):
                        nc.tensor.matmul(vtp, xt[:, c, ts_ * P:(ts_ + 1) * P], wv_sb[:, c],
                                         start=(c == 0), stop=(c == NC - 1))
                    nc.vector.tensor_copy(
                        v_sb[:, (4 * bk + ts_) * P:(4 * bk + ts_ + 1) * P], vtp)

            def attn(bk):
                nj = 4 * bk + 4
                for h in range(NREP):
                    cs_ps = pss.tile([1, TB], F32, tag="cs", bufs=2, name=f"cs{bk}_{h}")
                    ot_ps = pss.tile([P, TB], F32, tag="ot", bufs=2, name=f"ot{bk}_{h}")

                    def post(sp, j):
                        pt = ptpool.tile([P, TB], BF16, tag="pt", name=f"pt{bk}_{h}_{j}")
                        col = (h * NBLK + bk) * NJ + j
                        nc.scalar.activation(pt, sp, EXP, bias=cb_sb[:, col:col + 1])
                        delta = j - 4 * bk
                        src = pt
                        if delta >= 0:
                            ptm = ptpool.tile([P, TB], BF16, tag="ptm", name=f"ptm{bk}_{h}_{j}")
                            nc.vector.tensor_mul(ptm, pt, m01_sb[:, delta * TB:(delta + 1) * TB])
                            src = ptm
                        nc.tensor.matmul(cs_ps, ones_sb, src,
                                         start=(j == 0), stop=(j == nj - 1))
                        nc.tensor.matmul(ot_ps, v_sb[:, j * P:(j + 1) * P], src,
                                         start=(j == 0), stop=(j == nj - 1))

                    pend = None
                    for j in range(nj):
                        sp = pss.tile([P, TB], F32, tag="s", bufs=2, name=f"sp{bk}_{h}_{j}")
                        nc.tensor.matmul(sp, kT_sb[:, j * P:(j + 1) * P], qh_l[bk][h],
                                         start=True, stop=True)
                        if pend is not None:
                            post(*pend)
                        pend = (sp, j)
                    post(*pend)

                    rin = spool.tile([1, TB], F32, tag="rin", name=f"rin{bk}_{h}")
                    nc.vector.reciprocal_approx_fast(out=rin, in_=cs_ps)
                    rbc = spool.tile([P, TB], F32, tag="rbc", name=f"rbc{bk}_{h}")
                    nc.gpsimd.partition_broadcast(rbc, rin)
                    oh = opool.tile([P, TB], BF16, tag="oh", name=f"oh{bk}_{h}")
                    nc.vector.tensor_mul(oh, ot_ps, rbc)
                    oh_l[bk][h] = oh

            def wo_stage(bk):
                t0 = bk * TB
                for ts_ in range(4):
                    y_sb = ypool.tile([P, 4, TB], BF16, tag="ysb", name=f"y{bk}_{ts_}")
                    for e in range(4):
                        y_ps = pss.tile([P, TB], F32, tag="work", bufs=2, name=f"yp{bk}_{ts_}_{e}")
                        for h in range(NREP):
                            nc.tensor.matmul(y_ps, oh_l[bk][h][:, ts_ * P:(ts_ + 1) * P],
                                             wo_sb[:, h, e * TB:(e + 1) * TB],
                                             start=(h == 0), stop=(h == NREP - 1))
                        nc.vector.tensor_copy(y_sb[:, e], y_ps)
                    nc.sync.dma_start(
                        out=out[t0 + ts_ * P:t0 + (ts_ + 1) * P, :], in_=y_sb)

            # emission order = desired PE order (keeps PE dense / HAM warm)
            proj(0)
            proj(1)
            attn(0)
            proj(2)
            wo_stage(0)
            attn(1)
            proj(3)
            wo_stage(1)
            attn(2)
            wo_stage(2)
            attn(3)
            wo_stage(3)

    nc.compile()
    return nc


def _perm():
    """RoPE pair permutation: partner pairs (i, i+64) are placed 16 apart
    within the same 32-partition block so the DVE stream_shuffle (which only
    shuffles within 32-partition blocks) can do the partner swap."""
    perm = np.empty(HD, np.int64)
    for b in range(4):
        for s in range(16):
            perm[32 * b + s] = 16 * b + s
            perm[32 * b + 16 + s] = 64 + 16 * b + s
    newpos = np.arange(HD)
    sign_new = np.where(newpos % 32 < 16, -1.0, 1.0).astype(np.float32)
    return perm, sign_new


def _prep_inputs(x, mask, freqs_cis, alibi_bias, Wq, Wk, Wv, Wo):
    """Host-side prep: transposes, RoPE tables, ALiBi bias decomposition."""
    f64 = np.float64
    perm, sign_new = _perm()
    idx = np.arange(HD)
    cos_full = freqs_cis[:, idx // 2]                     # [T, 128]
    sin_full = freqs_cis[:, (HD // 2) + idx // 2]         # [T, 128]
    cosT = np.ascontiguousarray(cos_full.T[perm])         # [128, T], permuted
    sinT_signed = np.ascontiguousarray(sin_full.T[perm] * sign_new[:, None])

    cosq = (cosT * np.float32(SCALE)).astype(BFNP)
    sinq = (sinT_signed * np.float32(SCALE)).astype(BFNP)
    cosk = cosT.astype(BFNP)
    sink = sinT_signed.astype(BFNP)

    # permute rope feature columns of Wq (per q-head 128-block) and Wk
    Wq_p = Wq.reshape(D, H, HD)[:, :, perm].reshape(D, D)
    Wk_p = Wk.reshape(D, KV, HD)[:, :, perm].reshape(D, KV * HD)

    # multiplicative causal mask for the 4 diagonal key-tile offsets:
    # m01[p, delta*TB + f] = 1 if (128*delta + p) <= f else 0
    pvec = np.arange(P)[:, None]
    fvec = np.arange(TB)[None, :]
    m01 = np.empty((P, 4 * TB), np.float32)
    for delta in range(4):
        m01[:, delta * TB:(delta + 1) * TB] = (128 * delta + pvec <= fvec)
    m01 = m01.astype(BFNP)

    onesc = np.ones((P, 1), np.float32).astype(BFNP)

    in_maps = []
    for c in range(8):
        b, g = c // 4, c % 4
        slopes = np.array([-f64(alibi_bias[0, g * NREP + hl, 1, 0]) for hl in range(NREP)])
        pvec64 = np.arange(P, dtype=f64)
        jvec = np.arange(NJ, dtype=f64)
        # cb[p, h, bk, j] = ALIBI_W*slope*(j*128 + p) - ALIBI_W*slope*(bk*512 + 511)
        bkvec = np.arange(NBLK, dtype=f64)
        cbv = (ALIBI_W * slopes[:, None, None, None]
               * (jvec[None, None, :, None] * P + pvec64[None, None, None, :]
                  - (bkvec[None, :, None, None] * TB + (TB - 1))))
        cbm = np.ascontiguousarray(cbv.transpose(3, 0, 1, 2).reshape(P, NREP * NBLK * NJ)).astype(np.float32)
        in_maps.append({
            "xT": np.ascontiguousarray(x[b].T).astype(BFNP),
            "wq": np.ascontiguousarray(Wq_p[:, g * KVD:(g + 1) * KVD]).astype(BFNP),
            "wk": np.ascontiguousarray(Wk_p[:, g * P:(g + 1) * P]).astype(BFNP),
            "wv": np.ascontiguousarray(Wv[:, g * P:(g + 1) * P]).astype(BFNP),
            "wo": np.ascontiguousarray(Wo[g * KVD:(g + 1) * KVD, :]).astype(BFNP),
            "cosq": cosq, "sinq": sinq, "cosk": cosk, "sink": sink,
            "cb": cbm, "m01": m01,
            "onesc": onesc,
        })
    return in_maps


def kernel(x, mask, freqs_cis, alibi_bias, Wq, Wk, Wv, Wo, _trace=False, _trace_kwargs=None):
    from concourse.bass_utils import run_bass_kernel_spmd

    if "nc" not in _cache:
        _cache["nc"] = _build()
    nc = _cache["nc"]

    in_maps = _prep_inputs(np.asarray(x, np.float32), np.asarray(mask, np.float32),
                           np.asarray(freqs_cis, np.float32), np.asarray(alibi_bias, np.float32),
                           np.asarray(Wq, np.float32), np.asarray(Wk, np.float32),
                           np.asarray(Wv, np.float32), np.asarray(Wo, np.float32))
    kw = {}
    if _trace:
        kw = dict(trace=True, **(_trace_kwargs or {}))
    res = run_bass_kernel_spmd(nc, in_maps, list(range(8)), **kw)

    full = np.zeros((B, T, D), np.float32)
    for c in range(8):
        full[c // 4] += np.asarray(res.results[c]["out"], np.float32)
    if _trace:
        _cache["last_trace"] = res
    return full
